# revision 6
# baseline (speedup 1.0000x reference)
"""DiffGRUCell fused kernel for Trainium2 (Bass/Tile), 8-core data-parallel.

Computes, for x = reshape(diffused_x, (B*N, K*F)) and h = h_prev:
    z = sigmoid([x, h] @ Wz + bz)
    r = sigmoid([x, h] @ Wr + br)
    c = tanh([x, r*h] @ Wc + bc)
    h_new = (1 - z) * h + z * c

Sharding: data-parallel over batch. B*N = 20800 tokens are split into 8
contiguous shards of 2600 tokens (8 batches each); gate weights are
replicated. No collectives needed.

Per-core layout strategy:
  - Activations are transposed on-chip (TensorE + identity) into
    feature-major tiles actT[j] = combined.T tile of the 1216-row
    contraction dim: j=0 -> x[0:128], j=1 -> x[128:192] + bias-ones row,
    j=2+k -> h[128k:128(k+1)].
  - Gate biases are folded into the GEMM: weight c-tile j=1 carries the
    bias as an extra row, matched by a constant-1.0 row in actT[1].
  - r is computed feature-major ([h_tile, tokens], weights stationary) so
    rh = sigmoid(r) * h.T is immediately usable as the stationary operand
    of the candidate GEMM.
  - z and c are computed token-major ([tokens, h], activations stationary,
    weights moving) so the final elementwise combine with the
    naturally-loaded h_prev and the output store need no transposes.
"""

import numpy as np

import concourse.bass as bass
from concourse import bacc
import concourse.mybir as mybir
from concourse.tile import TileContext
from concourse.masks import make_identity
from concourse.bass_utils import run_bass_kernel_spmd

B, N, K, F, H = 64, 325, 3, 64, 1024
XW = K * F            # 192
CONCAT = XW + H       # 1216
NCORES = 8
TPC = (B * N) // NCORES   # 2600 tokens per core
TB = 256                  # token block size
F32 = mybir.dt.float32
MM_DT = mybir.dt.float32r  # matmul compute dtype (float32r: 4x faster PE)


def build(tpc=TPC, tb=TB, mm_dt=MM_DT):
    nc = bacc.Bacc("TRN2")
    x = nc.declare_dram_parameter("x", [tpc, XW], F32, isOutput=False)
    h = nc.declare_dram_parameter("h", [tpc, H], F32, isOutput=False)
    W = {}
    bvec = {}
    for g in "zrc":
        W[g] = nc.declare_dram_parameter(f"W{g}", [CONCAT, H], F32, isOutput=False)
        bvec[g] = nc.declare_dram_parameter(f"b{g}", [H], F32, isOutput=False)
    out = nc.declare_dram_parameter("out", [tpc, H], F32, isOutput=True)

    SIG = mybir.ActivationFunctionType.Sigmoid
    TANH = mybir.ActivationFunctionType.Tanh

    with TileContext(nc) as tc:
        with (
            tc.tile_pool(name="wpool", bufs=1) as wpool,
            tc.tile_pool(name="cpool", bufs=1) as cpool,
            tc.tile_pool(name="xnat", bufs=4) as xpool,
            tc.tile_pool(name="hnat", bufs=4) as hpool,
            tc.tile_pool(name="actT", bufs=20) as apool,
            tc.tile_pool(name="rh", bufs=8) as rhpool,
            tc.tile_pool(name="zsb", bufs=4) as zpool,
            tc.tile_pool(name="csb", bufs=4) as cbpool,
            tc.tile_pool(name="trps", bufs=2, space="PSUM") as trps,
            tc.tile_pool(name="rps", bufs=2, space="PSUM") as rps,
            tc.tile_pool(name="zcps", bufs=4, space="PSUM") as zcps,
        ):
            idt = cpool.tile([128, 128], F32, tag="idt")
            make_identity(nc, idt)
            ones = cpool.tile([1, tb], F32, tag="ones")
            nc.vector.memset(ones, 1.0)

            # Weight contraction tiles per gate; (tile, K) with bias row
            # appended to the j=1 (64-row) tile.
            wt = {}
            for g in "zrc":
                tiles = []
                t = wpool.tile([128, H], mm_dt, tag=f"w{g}0")
                nc.sync.dma_start(out=t, in_=W[g][0:128, :].bitcast(mm_dt))
                tiles.append((t, 128))
                t = wpool.tile([128, H], mm_dt, tag=f"w{g}1")
                nc.sync.dma_start(out=t[0:64, :], in_=W[g][128:192, :].bitcast(mm_dt))
                nc.sync.dma_start(
                    out=t[64:65, :], in_=bvec[g][:].unsqueeze(0).bitcast(mm_dt)
                )
                tiles.append((t, 65))
                for k in range(8):
                    t = wpool.tile([128, H], mm_dt, tag=f"w{g}h{k}")
                    nc.sync.dma_start(
                        out=t,
                        in_=W[g][XW + 128 * k : XW + 128 * (k + 1), :].bitcast(mm_dt),
                    )
                    tiles.append((t, 128))
                wt[g] = tiles

            nblocks = (tpc + tb - 1) // tb
            for bidx in range(nblocks):
                t0 = bidx * tb
                btb = min(tb, tpc - t0)
                nsub = (btb + 127) // 128

                xts = []
                hts = []
                for s in range(nsub):
                    r0 = t0 + s * 128
                    ts_ = min(128, t0 + btb - r0)
                    xt = xpool.tile([128, XW], F32, tag="xnat")
                    nc.sync.dma_start(out=xt[:ts_, :], in_=x[r0 : r0 + ts_, :])
                    ht = hpool.tile([128, H], F32, tag="hnat")
                    nc.sync.dma_start(out=ht[:ts_, :], in_=h[r0 : r0 + ts_, :])
                    xts.append((xt, ts_))
                    hts.append((ht, ts_))

                a = [apool.tile([128, tb], mm_dt, tag="actT", name=f"actT{bidx}_{i}") for i in range(10)]
                nc.scalar.copy(out=a[1][64:65, :btb], in_=ones[:, :btb])

                # Transpose x and h into feature-major actT tiles.
                for s in range(nsub):
                    xt, ts_ = xts[s]
                    ht, _ = hts[s]
                    srcs = [(xt, 0, 128, a[0]), (xt, 128, 64, a[1])]
                    srcs += [(ht, 128 * k, 128, a[2 + k]) for k in range(8)]
                    for src, c0, cw, dst in srcs:
                        pt = trps.tile([128, 128], F32, tag="trps")
                        nc.tensor.transpose(
                            pt[:cw, :ts_], src[:ts_, c0 : c0 + cw], idt[:ts_, :ts_]
                        )
                        nc.scalar.copy(
                            out=dst[0:cw, s * 128 : s * 128 + ts_], in_=pt[:cw, :ts_]
                        )

                # r gate, feature-major; rh[k] = sigmoid(r)[k] * h.T[k]
                rh = [rhpool.tile([128, tb], mm_dt, tag="rh", name=f"rh{bidx}_{i}") for i in range(8)]
                for k in range(8):
                    pr = rps.tile([128, tb], F32, tag="rps")
                    for j, (wtile, kk) in enumerate(wt["r"]):
                        nc.tensor.matmul(
                            pr[:, :btb],
                            lhsT=wtile[:kk, 128 * k : 128 * (k + 1)],
                            rhs=a[j][:kk, :btb],
                            start=(j == 0),
                            stop=(j == 9),
                        )
                    nc.scalar.activation(out=rh[k][:, :btb], in_=pr[:, :btb], func=SIG)
                    nc.vector.tensor_mul(
                        rh[k][:, :btb], rh[k][:, :btb], a[2 + k][:128, :btb]
                    )

                # z gate, token-major
                zts = []
                for s in range(nsub):
                    _, ts_ = xts[s]
                    zt = zpool.tile([128, H], F32, tag="zsb")
                    for hh in range(2):
                        pz = zcps.tile([128, 512], F32, tag="zcps")
                        for j, (wtile, kk) in enumerate(wt["z"]):
                            nc.tensor.matmul(
                                pz[:ts_, :],
                                lhsT=a[j][:kk, s * 128 : s * 128 + ts_],
                                rhs=wtile[:kk, 512 * hh : 512 * (hh + 1)],
                                start=(j == 0),
                                stop=(j == 9),
                            )
                        nc.scalar.activation(
                            out=zt[:ts_, 512 * hh : 512 * (hh + 1)],
                            in_=pz[:ts_, :],
                            func=SIG,
                        )
                    zts.append(zt)

                # c gate, token-major; h-part stationary operand comes from rh
                for s in range(nsub):
                    ht, ts_ = hts[s]
                    ct = cbpool.tile([128, H], F32, tag="csb")
                    for hh in range(2):
                        pc = zcps.tile([128, 512], F32, tag="zcps")
                        for j, (wtile, kk) in enumerate(wt["c"]):
                            lhs_src = a[j] if j < 2 else rh[j - 2]
                            nc.tensor.matmul(
                                pc[:ts_, :],
                                lhsT=lhs_src[:kk, s * 128 : s * 128 + ts_],
                                rhs=wtile[:kk, 512 * hh : 512 * (hh + 1)],
                                start=(j == 0),
                                stop=(j == 9),
                            )
                        nc.scalar.activation(
                            out=ct[:ts_, 512 * hh : 512 * (hh + 1)],
                            in_=pc[:ts_, :],
                            func=TANH,
                        )
                    # h_new = h + z*(c - h), computed in place in ct
                    r0 = t0 + s * 128
                    nc.vector.tensor_sub(ct[:ts_, :], ct[:ts_, :], ht[:ts_, :])
                    nc.vector.tensor_mul(ct[:ts_, :], ct[:ts_, :], zts[s][:ts_, :])
                    nc.vector.tensor_add(ct[:ts_, :], ct[:ts_, :], ht[:ts_, :])
                    nc.sync.dma_start(out=out[r0 : r0 + ts_, :], in_=ct[:ts_, :])

    nc.finalize()
    return nc


_NC_CACHE = {}


def _get_nc():
    key = (TPC, TB, str(MM_DT))
    if key not in _NC_CACHE:
        _NC_CACHE[key] = build()
    return _NC_CACHE[key]


def _make_in_maps(diffused_x, h_prev, Wz, bz, Wr, br, Wc, bc, tpc=TPC):
    x = np.ascontiguousarray(
        np.asarray(diffused_x, dtype=np.float32).reshape(B * N, XW)
    )
    hp = np.ascontiguousarray(np.asarray(h_prev, dtype=np.float32).reshape(B * N, H))
    shared = {
        "Wz": Wz, "bz": bz, "Wr": Wr, "br": br, "Wc": Wc, "bc": bc,
    }
    shared = {
        k: np.ascontiguousarray(np.asarray(v, dtype=np.float32))
        for k, v in shared.items()
    }
    in_maps = []
    for c in range(NCORES):
        sl = slice(c * tpc, (c + 1) * tpc)
        m = {"x": x[sl], "h": hp[sl]}
        m.update(shared)
        in_maps.append(m)
    return in_maps


def kernel(diffused_x, h_prev, Wz, bz, Wr, br, Wc, bc):
    nc = _get_nc()
    in_maps = _make_in_maps(diffused_x, h_prev, Wz, bz, Wr, br, Wc, bc)
    res = run_bass_kernel_spmd(nc, in_maps, list(range(NCORES)))
    outs = [res.results[c]["out"] for c in range(NCORES)]
    return np.concatenate(outs, axis=0).reshape(B, N, H)


def kernel_traced(diffused_x, h_prev, Wz, bz, Wr, br, Wc, bc):
    """Like kernel() but with NTFF profiling; returns (out, BassKernelResults)."""
    nc = _get_nc()
    in_maps = _make_in_maps(diffused_x, h_prev, Wz, bz, Wr, br, Wc, bc)
    res = run_bass_kernel_spmd(nc, in_maps, list(range(NCORES)), trace=True)
    outs = [res.results[c]["out"] for c in range(NCORES)]
    return np.concatenate(outs, axis=0).reshape(B, N, H), res


# revision 10
# speedup vs baseline: 1.1384x; 1.1384x over previous
"""DiffGRUCell fused kernel for Trainium2 (Bass/Tile), 8-core data-parallel.

Computes, for x = reshape(diffused_x, (B*N, K*F)) and h = h_prev:
    z = sigmoid([x, h] @ Wz + bz)
    r = sigmoid([x, h] @ Wr + br)
    c = tanh([x, r*h] @ Wc + bc)
    h_new = (1 - z) * h + z * c

Sharding: data-parallel over batch. B*N = 20800 tokens are split into 8
contiguous shards of 2600 tokens (8 batches each); gate weights are
replicated. No collectives needed.

Per-core layout strategy:
  - Activations are transposed on-chip (TensorE + identity) into
    feature-major tiles actT[j] = combined.T tile of the 1216-row
    contraction dim: j=0 -> x[0:128], j=1 -> x[128:192] + bias-ones row,
    j=2+k -> h[128k:128(k+1)].
  - Gate biases are folded into the GEMM: weight c-tile j=1 carries the
    bias as an extra row, matched by a constant-1.0 row in actT[1].
  - r is computed feature-major ([h_tile, tokens], weights stationary) so
    rh = sigmoid(r) * h.T is immediately usable as the stationary operand
    of the candidate GEMM.
  - z and c are computed token-major ([tokens, h], activations stationary,
    weights moving) so the final elementwise combine with the
    naturally-loaded h_prev and the output store need no transposes.
"""

import numpy as np

import concourse.bass as bass
from concourse import bacc
import concourse.mybir as mybir
from concourse.tile import TileContext
from concourse.masks import make_identity
from concourse.bass_utils import run_bass_kernel_spmd

B, N, K, F, H = 64, 325, 3, 64, 1024
XW = K * F            # 192
CONCAT = XW + H       # 1216
NCORES = 8
TPC = (B * N) // NCORES   # 2600 tokens per core
TB = 256                  # token block size
F32 = mybir.dt.float32
MM_DT = mybir.dt.float32r  # matmul compute dtype (float32r: 4x faster PE)


def build(tpc=TPC, tb=TB, mm_dt=MM_DT):
    nc = bacc.Bacc("TRN2")
    x = nc.declare_dram_parameter("x", [tpc, XW], F32, isOutput=False)
    h = nc.declare_dram_parameter("h", [tpc, H], F32, isOutput=False)
    W = {}
    bvec = {}
    for g in "zrc":
        W[g] = nc.declare_dram_parameter(f"W{g}", [CONCAT, H], F32, isOutput=False)
        bvec[g] = nc.declare_dram_parameter(f"b{g}", [H], F32, isOutput=False)
    out = nc.declare_dram_parameter("out", [tpc, H], F32, isOutput=True)

    SIG = mybir.ActivationFunctionType.Sigmoid
    TANH = mybir.ActivationFunctionType.Tanh

    with TileContext(nc) as tc:
        with (
            tc.tile_pool(name="wpool", bufs=1) as wpool,
            tc.tile_pool(name="cpool", bufs=1) as cpool,
            tc.tile_pool(name="xnat", bufs=6) as xpool,
            tc.tile_pool(name="hnat", bufs=6) as hpool,
            tc.tile_pool(name="actT", bufs=20) as apool,
            tc.tile_pool(name="rh", bufs=8) as rhpool,
            tc.tile_pool(name="zsb", bufs=3) as zpool,
            tc.tile_pool(name="csb", bufs=3) as cbpool,
            tc.tile_pool(name="trps", bufs=2, space="PSUM") as trps,
            tc.tile_pool(name="rps", bufs=2, space="PSUM") as rps,
            tc.tile_pool(name="zcps", bufs=4, space="PSUM") as zcps,
        ):
            idt = cpool.tile([128, 128], F32, tag="idt")
            make_identity(nc, idt)
            ones = cpool.tile([1, tb], F32, tag="ones")
            nc.vector.memset(ones, 1.0)

            # Block schedule: ragged block first (keeps the tail a full,
            # well-overlapped block), then the full blocks in order.
            nblocks = (tpc + tb - 1) // tb
            order = list(range(nblocks))
            if nblocks > 1 and tpc % tb != 0:
                order = [nblocks - 1] + order[:-1]

            def emit_loads(bidx):
                t0 = bidx * tb
                btb = min(tb, tpc - t0)
                nsub = (btb + 127) // 128
                xts = []
                hts = []
                for s in range(nsub):
                    r0 = t0 + s * 128
                    ts_ = min(128, t0 + btb - r0)
                    xt = xpool.tile([128, XW], F32, tag="xnat", name=f"xn{bidx}_{s}")
                    nc.sync.dma_start(out=xt[:ts_, :], in_=x[r0 : r0 + ts_, :])
                    ht = hpool.tile([128, H], F32, tag="hnat", name=f"hn{bidx}_{s}")
                    nc.sync.dma_start(out=ht[:ts_, :], in_=h[r0 : r0 + ts_, :])
                    xts.append((xt, ts_))
                    hts.append((ht, ts_))
                return xts, hts

            def emit_transposes(bidx, xts, hts):
                t0 = bidx * tb
                btb = min(tb, tpc - t0)
                nsub = (btb + 127) // 128
                a = [
                    apool.tile([128, tb], mm_dt, tag="actT", name=f"actT{bidx}_{i}")
                    for i in range(10)
                ]
                nc.scalar.copy(out=a[1][64:65, :btb], in_=ones[:, :btb])
                for s in range(nsub):
                    xt, ts_ = xts[s]
                    ht, _ = hts[s]
                    srcs = [(xt, 0, 128, a[0]), (xt, 128, 64, a[1])]
                    srcs += [(ht, 128 * k, 128, a[2 + k]) for k in range(8)]
                    for src, c0, cw, dst in srcs:
                        pt = trps.tile([128, 128], F32, tag="trps")
                        nc.tensor.transpose(
                            pt[:cw, :ts_], src[:ts_, c0 : c0 + cw], idt[:ts_, :ts_]
                        )
                        nc.scalar.copy(
                            out=dst[0:cw, s * 128 : s * 128 + ts_], in_=pt[:cw, :ts_]
                        )
                return a

            # Prologue: first two blocks' activations load + transpose before
            # any GEMM, so the PE has work while the 15MB of weights stream in.
            state = {}
            state[order[0]] = emit_loads(order[0])
            if len(order) > 1:
                state[order[1]] = emit_loads(order[1])

            # Weight contraction tiles per gate; (tile, K) with bias row
            # appended to the j=1 (64-row) tile. Emitted in first-use order
            # (r gate runs first in every block).
            wt = {}
            for g in "rzc":
                tiles = []
                t = wpool.tile([128, H], mm_dt, tag=f"w{g}0")
                nc.sync.dma_start(out=t, in_=W[g][0:128, :].bitcast(mm_dt))
                tiles.append((t, 128))
                t = wpool.tile([128, H], mm_dt, tag=f"w{g}1")
                nc.sync.dma_start(out=t[0:64, :], in_=W[g][128:192, :].bitcast(mm_dt))
                nc.sync.dma_start(
                    out=t[64:65, :], in_=bvec[g][:].unsqueeze(0).bitcast(mm_dt)
                )
                tiles.append((t, 65))
                for k in range(8):
                    t = wpool.tile([128, H], mm_dt, tag=f"w{g}h{k}")
                    nc.sync.dma_start(
                        out=t,
                        in_=W[g][XW + 128 * k : XW + 128 * (k + 1), :].bitcast(mm_dt),
                    )
                    tiles.append((t, 128))
                wt[g] = tiles

            acts = {}
            acts[order[0]] = emit_transposes(order[0], *state[order[0]])
            if len(order) > 1:
                acts[order[1]] = emit_transposes(order[1], *state[order[1]])

            for i, bidx in enumerate(order):
                if i + 2 < len(order):
                    state[order[i + 2]] = emit_loads(order[i + 2])

                t0 = bidx * tb
                btb = min(tb, tpc - t0)
                nsub = (btb + 127) // 128
                xts, hts = state[bidx]
                a = acts.pop(bidx)

                # r gate, feature-major; rh[k] = sigmoid(r)[k] * h.T[k]
                rh = [rhpool.tile([128, tb], mm_dt, tag="rh", name=f"rh{bidx}_{i}") for i in range(8)]
                for k in range(8):
                    pr = rps.tile([128, tb], F32, tag="rps")
                    for j, (wtile, kk) in enumerate(wt["r"]):
                        nc.tensor.matmul(
                            pr[:, :btb],
                            lhsT=wtile[:kk, 128 * k : 128 * (k + 1)],
                            rhs=a[j][:kk, :btb],
                            start=(j == 0),
                            stop=(j == 9),
                        )
                    nc.scalar.activation(out=rh[k][:, :btb], in_=pr[:, :btb], func=SIG)
                    nc.vector.tensor_mul(
                        rh[k][:, :btb], rh[k][:, :btb], a[2 + k][:128, :btb]
                    )

                # z gate, token-major
                zts = []
                for s in range(nsub):
                    _, ts_ = xts[s]
                    zt = zpool.tile([128, H], F32, tag="zsb")
                    for hh in range(2):
                        pz = zcps.tile([128, 512], F32, tag="zcps")
                        for j, (wtile, kk) in enumerate(wt["z"]):
                            nc.tensor.matmul(
                                pz[:ts_, :],
                                lhsT=a[j][:kk, s * 128 : s * 128 + ts_],
                                rhs=wtile[:kk, 512 * hh : 512 * (hh + 1)],
                                start=(j == 0),
                                stop=(j == 9),
                            )
                        nc.scalar.activation(
                            out=zt[:ts_, 512 * hh : 512 * (hh + 1)],
                            in_=pz[:ts_, :],
                            func=SIG,
                        )
                    zts.append(zt)

                # c gate, token-major; h-part stationary operand comes from rh
                for s in range(nsub):
                    ht, ts_ = hts[s]
                    ct = cbpool.tile([128, H], F32, tag="csb")
                    for hh in range(2):
                        pc = zcps.tile([128, 512], F32, tag="zcps")
                        for j, (wtile, kk) in enumerate(wt["c"]):
                            lhs_src = a[j] if j < 2 else rh[j - 2]
                            nc.tensor.matmul(
                                pc[:ts_, :],
                                lhsT=lhs_src[:kk, s * 128 : s * 128 + ts_],
                                rhs=wtile[:kk, 512 * hh : 512 * (hh + 1)],
                                start=(j == 0),
                                stop=(j == 9),
                            )
                        nc.scalar.activation(
                            out=ct[:ts_, 512 * hh : 512 * (hh + 1)],
                            in_=pc[:ts_, :],
                            func=TANH,
                        )
                    # h_new = h + z*(c - h), computed in place in ct
                    r0 = t0 + s * 128
                    nc.vector.tensor_sub(ct[:ts_, :], ct[:ts_, :], ht[:ts_, :])
                    nc.vector.tensor_mul(ct[:ts_, :], ct[:ts_, :], zts[s][:ts_, :])
                    nc.vector.tensor_add(ct[:ts_, :], ct[:ts_, :], ht[:ts_, :])
                    nc.sync.dma_start(out=out[r0 : r0 + ts_, :], in_=ct[:ts_, :])

                if i + 2 < len(order):
                    acts[order[i + 2]] = emit_transposes(
                        order[i + 2], *state[order[i + 2]]
                    )

    nc.finalize()
    return nc


_NC_CACHE = {}


def _get_nc():
    key = (TPC, TB, str(MM_DT))
    if key not in _NC_CACHE:
        _NC_CACHE[key] = build()
    return _NC_CACHE[key]


def _make_in_maps(diffused_x, h_prev, Wz, bz, Wr, br, Wc, bc, tpc=TPC):
    x = np.ascontiguousarray(
        np.asarray(diffused_x, dtype=np.float32).reshape(B * N, XW)
    )
    hp = np.ascontiguousarray(np.asarray(h_prev, dtype=np.float32).reshape(B * N, H))
    shared = {
        "Wz": Wz, "bz": bz, "Wr": Wr, "br": br, "Wc": Wc, "bc": bc,
    }
    shared = {
        k: np.ascontiguousarray(np.asarray(v, dtype=np.float32))
        for k, v in shared.items()
    }
    in_maps = []
    for c in range(NCORES):
        sl = slice(c * tpc, (c + 1) * tpc)
        m = {"x": x[sl], "h": hp[sl]}
        m.update(shared)
        in_maps.append(m)
    return in_maps


def kernel(diffused_x, h_prev, Wz, bz, Wr, br, Wc, bc):
    nc = _get_nc()
    in_maps = _make_in_maps(diffused_x, h_prev, Wz, bz, Wr, br, Wc, bc)
    res = run_bass_kernel_spmd(nc, in_maps, list(range(NCORES)))
    outs = [res.results[c]["out"] for c in range(NCORES)]
    return np.concatenate(outs, axis=0).reshape(B, N, H)


def kernel_traced(diffused_x, h_prev, Wz, bz, Wr, br, Wc, bc):
    """Like kernel() but with NTFF profiling; returns (out, BassKernelResults)."""
    nc = _get_nc()
    in_maps = _make_in_maps(diffused_x, h_prev, Wz, bz, Wr, br, Wc, bc)
    res = run_bass_kernel_spmd(nc, in_maps, list(range(NCORES)), trace=True)
    outs = [res.results[c]["out"] for c in range(NCORES)]
    return np.concatenate(outs, axis=0).reshape(B, N, H), res
